# revision 1
# baseline (speedup 1.0000x reference)
"""Trainium2 Bass kernel for nn_CrossAttention (B=2, N=2048, C=1024, H=16, D=64).

Sharding: 8 cores = 2 batches x 4 head-groups (4 heads each).
Each core computes its head-group's attention + a partial output projection;
the host sums the 4 partials per batch and adds the bias.

Device pipeline per core:
  P1: q/k/v projections (f32r matmuls, normal layout), zero-mean folded into
      host-centered weights, variance via ACT Square + DVE reduce, rstd scale +
      RoPE fused on DVE (bf16), PE transposes into head-paired q^T/k^T tiles,
      gate projected in transposed layout (raw, sigmoid deferred).
  P2: per q-block: row-tiled paired score matmuls (K=64 x2 concurrent), ACT exp
      straight from 2-bank PSUM (scale=1/8; no max subtraction - rms-normed
      q,k bound |score| <= 8), col-tiled paired attn@v + M=1 ones matmuls for
      softmax denominators, tanh-based sigmoid gating x reciprocal-denominator
      broadcast, bf16 output projection, f32 partial out.
"""

import os
import sys
import numpy as np

for _p in ("/opt/trn_rl_repo", "/opt/pypackages"):
    if _p not in sys.path:
        sys.path.insert(0, _p)

B, N, C = 2, 2048, 1024
H, D = 16, 64
HG = 4            # heads per core
NCH = 16          # token chunks of 128
QB = 4            # q blocks of 512
KTC = 16          # key chunks of 128
EPS = 1e-6

_PROG = None      # cached compiled Bass program
LAST_EXEC_NS = None
LAST_PROFILE = None


def _build_program():
    import concourse.bass as bass
    import concourse.bacc as bacc
    import concourse.tile as tile
    import concourse.mybir as mybir

    F32 = mybir.dt.float32
    F32R = mybir.dt.float32r
    BF = mybir.dt.bfloat16
    AF = mybir.ActivationFunctionType
    OP = mybir.AluOpType

    nc = bacc.Bacc("TRN2", target_bir_lowering=False, debug=False, num_devices=8)

    xT = nc.dram_tensor("xT", [8, 128, N], F32R, kind="ExternalInput")
    ctxT = nc.dram_tensor("ctxT", [8, 128, N], F32R, kind="ExternalInput")
    wq = nc.dram_tensor("wq", [8, 128, 256], F32R, kind="ExternalInput")
    wg = nc.dram_tensor("wg", [8, 128, 256], F32R, kind="ExternalInput")
    wkv = nc.dram_tensor("wkv", [8, 128, 512], F32R, kind="ExternalInput")
    wo = nc.dram_tensor("wo", [2, 128, 1024], BF, kind="ExternalInput")
    cosq = nc.dram_tensor("cosq", [N, D], F32, kind="ExternalInput")
    ssinq = nc.dram_tensor("ssinq", [N, D], F32, kind="ExternalInput")
    cosk = nc.dram_tensor("cosk", [N, D], F32, kind="ExternalInput")
    ssink = nc.dram_tensor("ssink", [N, D], F32, kind="ExternalInput")
    part = nc.dram_tensor("part", [N, C], F32, kind="ExternalOutput")
    debug = bool(os.environ.get("BASS_KERNEL_DEBUG"))
    if debug:
        dbg = {
            nm: nc.dram_tensor(f"dbg_{nm}", shp, BF, kind="ExternalOutput")
            for nm, shp in [
                ("pairQ0", [128, N]), ("pairQ1", [128, N]),
                ("pairK0", [128, N]), ("pairK1", [128, N]),
                ("v", [128, KTC, 4, 64]), ("graw", [128, 2, N]),
                ("A", [128, 2, N]),
                ("ssum", [128, 4]), ("rstd", [128, 4]),
                ("qn", [128, 4, 64]), ("qr", [128, 4, 64]),
                ("t1", [128, 4, 64]), ("t2", [128, 4, 64]),
            ]
        }

    def bcast4(ap):
        # [128, 64] -> [128, 4, 64] with step-0 middle dim (read-broadcast)
        return bass.AP(tensor=ap.tensor, offset=ap.offset,
                       ap=[ap.ap[0], [0, 4], ap.ap[1]])

    def swap_view(ap):
        # ap: [128, 4, 64] contiguous -> per head read order d+32..d+63, d..d+31
        p, hdim, ddim = ap.ap
        return bass.AP(tensor=ap.tensor, offset=ap.offset + 32 * ddim[0],
                       ap=[p, hdim, [-32 * ddim[0], 2], [ddim[0], 32]])

    with tile.TileContext(nc) as tc:
        import contextlib
        with contextlib.ExitStack() as ctx:
            singles = ctx.enter_context(tc.tile_pool(name="singles", bufs=1))
            slices = ctx.enter_context(tc.tile_pool(name="slices", bufs=2))
            work = ctx.enter_context(tc.tile_pool(name="work", bufs=2))
            persist = ctx.enter_context(tc.tile_pool(name="persist", bufs=1))
            exps_p = ctx.enter_context(tc.tile_pool(name="exps", bufs=6))
            gat_p = ctx.enter_context(tc.tile_pool(name="gat", bufs=2))

            # ---- constants / weights ----
            wq_sb = singles.tile([128, 8, 256], F32R)
            nc.sync.dma_start(out=wq_sb, in_=wq.ap().rearrange("c p f -> p c f"))
            wg_sb = singles.tile([128, 8, 256], F32R)
            nc.sync.dma_start(out=wg_sb, in_=wg.ap().rearrange("c p f -> p c f"))
            wkv_sb = singles.tile([128, 8, 512], F32R)
            nc.sync.dma_start(out=wkv_sb, in_=wkv.ap().rearrange("c p f -> p c f"))
            wo_sb = singles.tile([128, 2, 1024], BF)
            nc.sync.dma_start(out=wo_sb, in_=wo.ap().rearrange("c p f -> p c f"))
            cq_sb = singles.tile([128, NCH, D], F32)
            nc.sync.dma_start(out=cq_sb, in_=cosq.ap().rearrange("(i p) d -> p i d", p=128))
            sq_sb = singles.tile([128, NCH, D], F32)
            nc.sync.dma_start(out=sq_sb, in_=ssinq.ap().rearrange("(i p) d -> p i d", p=128))
            ck_sb = singles.tile([128, NCH, D], F32)
            nc.sync.dma_start(out=ck_sb, in_=cosk.ap().rearrange("(i p) d -> p i d", p=128))
            sk_sb = singles.tile([128, NCH, D], F32)
            nc.sync.dma_start(out=sk_sb, in_=ssink.ap().rearrange("(i p) d -> p i d", p=128))
            from concourse.masks import make_identity
            ident = singles.tile([128, 128], F32)
            make_identity(nc, ident)
            ones1 = singles.tile([128, 1], BF)
            nc.vector.memset(ones1, 1.0)
            ones2 = singles.tile([128, 64], BF)
            nc.vector.memset(ones2, 1.0)
            eps_sb = singles.tile([128, 1], F32)
            nc.vector.memset(eps_sb, EPS)

            # ---- persistent intermediates ----
            pairQ = [persist.tile([128, N], BF, tag=f"pairQ{p}", name=f"pairQ{p}") for p in range(2)]
            pairK = [persist.tile([128, N], BF, tag=f"pairK{p}", name=f"pairK{p}") for p in range(2)]
            v_sb = persist.tile([128, KTC, 4, 64], BF, tag="v_sb")
            graw = persist.tile([128, 2, N], BF, tag="graw")
            A_sb = persist.tile([128, 2, N], BF, tag="A_sb")

            # ================= P1: projections / norm / rope / transposes ====
            with tc.tile_pool(name="psA", bufs=2, space="PSUM") as psA, \
                 tc.tile_pool(name="psT", bufs=2, space="PSUM") as psT:

                def qk_path(side, sl, ns, i, w_rhs, wcols, cos_t, sin_t, dst_pair):
                    """Project+norm+rope+transpose chunk i of q (side=0) or k."""
                    ps = psA.tile([128, 512], F32, tag="proj")
                    for c in range(8):
                        nc.tensor.matmul(ps[:, :wcols],
                                         sl[:, c, ns * 128:(ns + 1) * 128],
                                         w_rhs(c),
                                         start=(c == 0), stop=(c == 7))
                    qpart = ps[:, 0:256]
                    # variance (zero-mean folded into weights)
                    sqv = work.tile([128, 256], F32, tag="sq")
                    nc.scalar.activation(out=sqv, in_=qpart, func=AF.Square)
                    ssum = work.tile([128, 4], F32, tag="ssum")
                    nc.vector.tensor_reduce(
                        out=ssum, in_=sqv.rearrange("p (h d) -> p h d", h=4),
                        axis=mybir.AxisListType.X, op=OP.add)
                    sdev = work.tile([128, 4], F32, tag="sdev")
                    nc.scalar.activation(out=sdev, in_=ssum, func=AF.Sqrt,
                                         scale=1.0 / 64.0, bias=eps_sb)
                    rstd = work.tile([128, 4], F32, tag="rstd")
                    nc.vector.reciprocal(out=rstd, in_=sdev)
                    qn = work.tile([128, 4, 64], F32, tag="qn")
                    for h in range(4):
                        nc.vector.tensor_scalar_mul(
                            out=qn[:, h, :], in0=qpart[:, h * 64:(h + 1) * 64],
                            scalar1=rstd[:, h:h + 1])
                    # rope: qr = qn*cos + swap(qn)*ssin   (sign folded in ssin)
                    t1 = work.tile([128, 4, 64], F32, tag="t1")
                    nc.vector.tensor_tensor(out=t1, in0=qn, in1=bcast4(cos_t),
                                            op=OP.mult)
                    t2 = work.tile([128, 4, 64], F32, tag="t2")
                    nc.vector.tensor_tensor(out=t2, in0=swap_view(qn),
                                            in1=bcast4(sin_t), op=OP.mult)
                    qr = work.tile([128, 4, 64], F32, tag="qr")
                    nc.vector.tensor_tensor(out=qr, in0=t1, in1=t2, op=OP.add)
                    if debug and side == 0 and i == 5:
                        nc.sync.dma_start(out=dbg["ssum"].ap(),
                                          in_=ssum.bitcast(BF)[:, 1:8:2])
                        nc.sync.dma_start(out=dbg["rstd"].ap(),
                                          in_=rstd.bitcast(BF)[:, 1:8:2])
                        nc.gpsimd.dma_start(out=dbg["qn"].ap(), in_=qn)
                        nc.gpsimd.dma_start(out=dbg["qr"].ap(), in_=qr)
                        nc.gpsimd.dma_start(out=dbg["t1"].ap(), in_=t1)
                        nc.gpsimd.dma_start(out=dbg["t2"].ap(), in_=t2)
                    # PE transpose: heads (2p, 2p+1) -> pair tile slice (bf16)
                    for p in range(2):
                        pst = psT.tile([128, 128], F32, tag="tp")
                        nc.tensor.transpose(
                            pst,
                            qr[:, 2 * p:2 * p + 2, :].rearrange("p a b -> p (a b)"),
                            ident)
                        nc.vector.tensor_copy(
                            out=dst_pair[p][:, i * 128:(i + 1) * 128], in_=pst)
                    return ps

                # K/V path over all 16 chunks
                for qc in range(4):
                    c_sl = slices.tile([128, 8, 512], F32R, tag="slice")
                    nc.sync.dma_start(
                        out=c_sl,
                        in_=ctxT.ap()[:, :, qc * 512:(qc + 1) * 512]
                        .rearrange("c p n -> p c n"))
                    for ns in range(4):
                        j = qc * 4 + ns
                        ps = qk_path(1, c_sl, ns, j,
                                     lambda c: wkv_sb[:, c, :], 512,
                                     ck_sb[:, j, :], sk_sb[:, j, :], pairK)
                        # v evac (+ ones col already memset)
                        nc.vector.tensor_copy(
                            out=v_sb[:, j, :, 0:64],
                            in_=ps[:, 256:512].rearrange("p (h d) -> p h d", h=4))

                # Q path + raw gate over all 16 chunks
                for qc in range(4):
                    x_sl = slices.tile([128, 8, 512], F32R, tag="slice")
                    nc.sync.dma_start(
                        out=x_sl,
                        in_=xT.ap()[:, :, qc * 512:(qc + 1) * 512]
                        .rearrange("c p n -> p c n"))
                    for ns in range(4):
                        i = qc * 4 + ns
                        qk_path(0, x_sl, ns, i,
                                lambda c: wq_sb[:, c, :], 256,
                                cq_sb[:, i, :], sq_sb[:, i, :], pairQ)
                    # gate projection, transposed layout, raw (sigmoid later)
                    for gfc in range(2):
                        psg = psA.tile([128, 512], F32, tag="proj")
                        for c in range(8):
                            nc.tensor.matmul(
                                psg, wg_sb[:, c, gfc * 128:(gfc + 1) * 128],
                                x_sl[:, c, :], start=(c == 0), stop=(c == 7))
                        nc.scalar.activation(
                            out=graw[:, gfc, qc * 512:(qc + 1) * 512], in_=psg,
                            func=AF.Copy)

            # ================= P2: attention + gating + out-proj =============
            with tc.tile_pool(name="psSC", bufs=2, space="PSUM") as psSC, \
                 tc.tile_pool(name="psAO", bufs=2, space="PSUM") as psAO, \
                 tc.tile_pool(name="psDN", bufs=2, space="PSUM") as psDN:

                for qc in range(4):
                    qsl = slice(qc * 512, (qc + 1) * 512)
                    ao = []
                    dn = []
                    for p in range(2):
                        ao_p = psAO.tile([128, 512], F32, tag="ao")
                        dn_p = psDN.tile([128, 512], F32, tag="dn")
                        ao.append(ao_p)
                        dn.append(dn_p)
                        pend = []  # (expS tile, ktc) awaiting attn MMs

                        def flush_attn(eS, k):
                            # first MM of the k==0 group clears the whole bank;
                            # the second must NOT re-clear (would drop the
                            # first's has_written bits) -> start only on MM1.
                            st = (k == 0)
                            sp = (k == KTC - 1)
                            nc.tensor.matmul(ao_p[0:64, :], v_sb[:, k, 2 * p, :],
                                             eS[:, 0, :], start=st, stop=sp,
                                             tile_position=(0, 0),
                                             skip_group_check=True)
                            nc.tensor.matmul(ao_p[64:128, :], v_sb[:, k, 2 * p + 1, :],
                                             eS[:, 1, :], start=st, stop=sp,
                                             tile_position=(0, 64),
                                             skip_group_check=True)
                            nc.tensor.matmul(dn_p[0:1, :], ones1, eS[:, 0, :],
                                             start=st, stop=sp,
                                             tile_position=(0, 0),
                                             skip_group_check=True)
                            nc.tensor.matmul(dn_p[32:33, :], ones1, eS[:, 1, :],
                                             start=st, stop=sp,
                                             tile_position=(0, 32),
                                             skip_group_check=True)

                        for k in range(KTC):
                            ksl = slice(k * 128, (k + 1) * 128)
                            ps = psSC.tile([128, 1024], F32, tag="sc")
                            nc.tensor.matmul(ps[:, 0:512],
                                             pairK[p][0:64, ksl],
                                             pairQ[p][0:64, qsl],
                                             start=True, stop=True,
                                             tile_position=(0, 0))
                            nc.tensor.matmul(ps[:, 512:1024],
                                             pairK[p][64:128, ksl],
                                             pairQ[p][64:128, qsl],
                                             start=True, stop=True,
                                             tile_position=(64, 0))
                            eS = exps_p.tile([128, 2, 512], BF, tag="expS")
                            nc.scalar.activation(
                                out=eS.rearrange("p a b -> p (a b)"), in_=ps,
                                func=AF.Exp, scale=0.125)
                            pend.append((eS, k))
                            if len(pend) > 2:
                                flush_attn(*pend.pop(0))
                        for e in pend:
                            flush_attn(*e)

                    # gating per pair
                    for p in range(2):
                        gs = gat_p.tile([128, 512], F32, tag="gs")
                        nc.scalar.activation(out=gs, in_=graw[:, p, qsl],
                                             func=AF.Tanh, scale=0.5)
                        sig = gat_p.tile([128, 512], F32, tag="sig")
                        nc.vector.tensor_scalar(out=sig, in0=gs, scalar1=0.5,
                                                scalar2=0.5, op0=OP.mult,
                                                op1=OP.add)
                        dns = gat_p.tile([128, 512], BF, tag="dns")
                        nc.vector.tensor_copy(out=dns[0:1, :], in_=dn[p][0:1, :])
                        nc.vector.tensor_copy(out=dns[32:33, :],
                                              in_=dn[p][32:33, :])
                        rbc = psDN.tile([128, 512], F32, tag="dn")
                        nc.tensor.matmul(rbc[0:64, :], ones2[0:1, :], dns[0:1, :],
                                         start=True, stop=True,
                                         tile_position=(0, 0))
                        nc.tensor.matmul(rbc[64:128, :], ones2[32:33, :],
                                         dns[32:33, :], start=True, stop=True,
                                         tile_position=(32, 64))
                        rec = gat_p.tile([128, 512], F32, tag="rec")
                        nc.vector.reciprocal(out=rec, in_=rbc)
                        m = gat_p.tile([128, 512], F32, tag="m")
                        nc.vector.tensor_tensor(out=m, in0=sig, in1=rec,
                                                op=OP.mult)
                        nc.vector.tensor_tensor(out=A_sb[:, p, qsl], in0=ao[p],
                                                in1=m, op=OP.mult)

                    # output projection for this q block
                    for nk in range(4):
                        n1 = qc * 4 + nk
                        for oc in range(2):
                            po = psAO.tile([128, 512], F32, tag="ao")
                            for fc in range(2):
                                nc.tensor.matmul(
                                    po,
                                    A_sb[:, fc, n1 * 128:(n1 + 1) * 128],
                                    wo_sb[:, fc, oc * 512:(oc + 1) * 512],
                                    start=(fc == 0), stop=(fc == 1))
                            ev = gat_p.tile([128, 512], F32, tag="ev")
                            nc.scalar.activation(out=ev, in_=po, func=AF.Copy)
                            nc.sync.dma_start(
                                out=part.ap()[n1 * 128:(n1 + 1) * 128,
                                              oc * 512:(oc + 1) * 512],
                                in_=ev)

            if debug:
                for nm, src in [("pairQ0", pairQ[0]), ("pairQ1", pairQ[1]),
                                ("pairK0", pairK[0]), ("pairK1", pairK[1]),
                                ("v", v_sb), ("graw", graw), ("A", A_sb)]:
                    nc.sync.dma_start(out=dbg[nm].ap(), in_=src)

    nc.compile()
    return nc


def _prep_core(inputs, b, g, bf16):
    x = np.asarray(inputs["x"][b], dtype=np.float32)
    ctx = np.asarray(inputs["context"][b], dtype=np.float32)
    Wq = np.asarray(inputs["Wq"], dtype=np.float32).reshape(H, 2 * D, C)
    Wkv = np.asarray(inputs["Wkv"], dtype=np.float32).reshape(H, 2 * D, C)
    Wo = np.asarray(inputs["Wo"], dtype=np.float32)
    cos = np.asarray(inputs["cos"][b], dtype=np.float32)
    sin = np.asarray(inputs["sin"][b], dtype=np.float32)
    qw = np.asarray(inputs["q_norm_w"], dtype=np.float32)
    kw = np.asarray(inputs["k_norm_w"], dtype=np.float32)

    hs = slice(HG * g, HG * g + HG)
    qr = Wq[hs, :D, :]                       # [4, D, C]
    qr = qr - qr.mean(axis=1, keepdims=True)
    gr = Wq[hs, D:, :]
    kr = Wkv[hs, :D, :]
    kr = kr - kr.mean(axis=1, keepdims=True)
    vr = Wkv[hs, D:, :]

    sgn = np.where(np.arange(D) < D // 2, -1.0, 1.0).astype(np.float32)
    wswap = lambda w: np.concatenate([w[D // 2:], w[:D // 2]])

    return {
        "xT": np.ascontiguousarray(x.T).reshape(8, 128, N),
        "ctxT": np.ascontiguousarray(ctx.T).reshape(8, 128, N),
        "wq": np.ascontiguousarray(qr.reshape(HG * D, C).T).reshape(8, 128, 256),
        "wg": np.ascontiguousarray(gr.reshape(HG * D, C).T).reshape(8, 128, 256),
        "wkv": np.ascontiguousarray(
            np.concatenate([kr.reshape(HG * D, C), vr.reshape(HG * D, C)], 0).T
        ).reshape(8, 128, 512),
        "wo": np.ascontiguousarray(
            Wo[:, 256 * g:256 * (g + 1)].T).reshape(2, 128, C).astype(bf16),
        "cosq": (cos * qw[None, :]).astype(np.float32),
        "ssinq": (sin * sgn[None, :] * wswap(qw)[None, :]).astype(np.float32),
        "cosk": (cos * kw[None, :]).astype(np.float32),
        "ssink": (sin * sgn[None, :] * wswap(kw)[None, :]).astype(np.float32),
    }


def kernel(**inputs):
    global _PROG, LAST_EXEC_NS, LAST_PROFILE
    import ml_dtypes
    bf16 = ml_dtypes.bfloat16

    if _PROG is None:
        _PROG = _build_program()
    nc = _PROG

    in_maps = [_prep_core(inputs, core // 4, core % 4, bf16) for core in range(8)]

    trace = bool(os.environ.get("BASS_KERNEL_TRACE"))
    kw = {}
    if trace:
        import types
        from trn_agent_boot.trn_boot import _ntff_profile_via_ctypes
        hook = _ntff_profile_via_ctypes('/opt/axon/libaxon_pjrt.so')
        mod = types.ModuleType('antenv.axon_hooks')
        mod.get_axon_ntff_profile_hook = lambda: hook
        sys.modules['antenv.axon_hooks'] = mod
        from concourse import bass_utils
        bass_utils.upload_artifacts = lambda tmpdir: tmpdir
        kw = dict(trace=True, tmpdir=os.environ.get("BASS_KERNEL_TRACE_DIR"))

    from concourse.bass_utils import run_bass_kernel_spmd
    res = run_bass_kernel_spmd(nc, in_maps, core_ids=list(range(8)), **kw)
    LAST_EXEC_NS = res.exec_time_ns
    LAST_PROFILE = res.profile_json

    bo = np.asarray(inputs["bo"], dtype=np.float32)
    out = np.zeros((B, N, C), dtype=np.float32)
    for core in range(8):
        out[core // 4] += res.results[core]["part"]
    out += bo[None, None, :]
    return out



# revision 4
# speedup vs baseline: 1.1957x; 1.1957x over previous
"""Trainium2 Bass kernel for nn_CrossAttention (B=2, N=2048, C=1024, H=16, D=64).

Sharding: 8 cores = 2 batches x 4 head-groups (4 heads each).
Each core computes its head-group's attention + a partial output projection;
the host sums the 4 partials per batch and adds the bias.

v2 layout (all-bf16 matmul inputs, fp32 PSUM accumulation):
  P1: q/k/v projections (bf16 matmuls), zero-mean folded into host-centered
      weights, variance via ACT Square+accum_out, rstd on DVE, rope on DVE in
      bf16, PE transposes (bf16) into head-paired q^T/k^T tiles with ACT-copy
      evacuation, gate projected in transposed layout (raw, sigmoid deferred).
  P2: per q-block x head-pair: row-tiled paired score matmuls, ACT exp from
      2-bank PSUM (scale=1/8), col-tiled paired attn@v + M=1 ones matmuls for
      softmax denominators, gating via exp-sigmoid (same ACT table as the
      attention exp), reciprocal-denominator broadcast by PE, bf16 output
      projection interleaved into the NEXT q-block's attention to keep PE warm
      (HAM K=8/8), bf16 partial out.
"""

import os
import sys
import numpy as np

for _p in ("/opt/trn_rl_repo", "/opt/pypackages"):
    if _p not in sys.path:
        sys.path.insert(0, _p)

B, N, C = 2, 2048, 1024
H, D = 16, 64
HG = 4            # heads per core
NCH = 16          # token chunks of 128
QB = 4            # q blocks of 512
KTC = 16          # key chunks of 128
EPS = 1e-6

_PROG = None      # cached compiled Bass program
LAST_EXEC_NS = None
LAST_PROFILE = None


def _build_program():
    import concourse.bass as bass
    import concourse.bacc as bacc
    import concourse.tile as tile
    import concourse.mybir as mybir

    F32 = mybir.dt.float32
    BF = mybir.dt.bfloat16
    AF = mybir.ActivationFunctionType
    OP = mybir.AluOpType

    nc = bacc.Bacc("TRN2", target_bir_lowering=False, debug=False, num_devices=8)

    xT = nc.dram_tensor("xT", [8, 128, N], BF, kind="ExternalInput")
    ctxT = nc.dram_tensor("ctxT", [8, 128, N], BF, kind="ExternalInput")
    wq = nc.dram_tensor("wq", [8, 128, 256], BF, kind="ExternalInput")
    wg = nc.dram_tensor("wg", [8, 128, 256], BF, kind="ExternalInput")
    wkv = nc.dram_tensor("wkv", [8, 128, 512], BF, kind="ExternalInput")
    wo = nc.dram_tensor("wo", [2, 128, 1024], BF, kind="ExternalInput")
    cosq = nc.dram_tensor("cosq", [N, D], BF, kind="ExternalInput")
    ssinq = nc.dram_tensor("ssinq", [N, D], BF, kind="ExternalInput")
    cosk = nc.dram_tensor("cosk", [N, D], BF, kind="ExternalInput")
    ssink = nc.dram_tensor("ssink", [N, D], BF, kind="ExternalInput")
    part = nc.dram_tensor("part", [N, C], BF, kind="ExternalOutput")

    def bcast4(ap):
        # [128, 64] -> [128, 4, 64] with step-0 middle dim (read-broadcast)
        return bass.AP(tensor=ap.tensor, offset=ap.offset,
                       ap=[ap.ap[0], [0, 4], ap.ap[1]])

    def bcast_inner(ap, n):
        # [128, 4] -> [128, 4, n] with step-0 inner dim
        return bass.AP(tensor=ap.tensor, offset=ap.offset,
                       ap=[ap.ap[0], ap.ap[1], [0, n]])

    def swap_view(ap):
        # ap: [128, 4, 64] contiguous -> per head read order d+32..d+63, d..d+31
        p, hdim, ddim = ap.ap
        return bass.AP(tensor=ap.tensor, offset=ap.offset + 32 * ddim[0],
                       ap=[p, hdim, [-32 * ddim[0], 2], [ddim[0], 32]])

    with tile.TileContext(nc) as tc:
        import contextlib
        with contextlib.ExitStack() as ctx:
            singles = ctx.enter_context(tc.tile_pool(name="singles", bufs=1))
            slices = ctx.enter_context(tc.tile_pool(name="slices", bufs=2))
            work = ctx.enter_context(tc.tile_pool(name="work", bufs=2))
            persist = ctx.enter_context(tc.tile_pool(name="persist", bufs=1))
            exps_p = ctx.enter_context(tc.tile_pool(name="exps", bufs=6))
            gat_p = ctx.enter_context(tc.tile_pool(name="gat", bufs=2))

            # ---- K-side constants / weights first (needed earliest) ----
            ck_sb = singles.tile([128, NCH, D], BF)
            nc.sync.dma_start(out=ck_sb, in_=cosk.ap().rearrange("(i p) d -> p i d", p=128))
            sk_sb = singles.tile([128, NCH, D], BF)
            nc.sync.dma_start(out=sk_sb, in_=ssink.ap().rearrange("(i p) d -> p i d", p=128))
            wkv_sb = singles.tile([128, 8, 512], BF)
            nc.sync.dma_start(out=wkv_sb, in_=wkv.ap().rearrange("c p f -> p c f"))

            from concourse.masks import make_identity
            identb = singles.tile([128, 128], BF)
            make_identity(nc, identb)
            ones1 = singles.tile([128, 1], BF)
            nc.vector.memset(ones1, 1.0)
            ones2 = singles.tile([128, 64], BF)
            nc.vector.memset(ones2, 1.0)
            eps_sb = singles.tile([128, 1], F32)
            nc.vector.memset(eps_sb, EPS)
            warm = singles.tile([128, 512], BF)
            nc.vector.memset(warm, 0.0)

            # ---- persistent intermediates ----
            pairQ = [persist.tile([128, N], BF, tag=f"pairQ{p}", name=f"pairQ{p}") for p in range(2)]
            pairK = [persist.tile([128, N], BF, tag=f"pairK{p}", name=f"pairK{p}") for p in range(2)]
            v_sb = persist.tile([128, KTC, 4, 64], BF, tag="v_sb")
            graw = persist.tile([128, 2, N], BF, tag="graw")
            A_sb = persist.tile([128, 2, N], BF, tag="A_sb")

            # ================= P1: projections / norm / rope / transposes ====
            with tc.tile_pool(name="psA", bufs=2, space="PSUM") as psA, \
                 tc.tile_pool(name="psT", bufs=2, space="PSUM") as psT:

                # PE warmup while input DMAs are in flight: keeps HAM busy so
                # the real matmuls start at K=8/8 (2.4 GHz) instead of 1.2.
                for _ in range(14):
                    wps = psA.tile([128, 512], F32, tag="proj", name="wps")
                    nc.tensor.matmul(wps, warm[:, 0:128], warm,
                                     start=True, stop=True)

                pend_tp = []  # transposes deferred one chunk to keep PE fed

                def flush_tp():
                    for (qr_t, dst_pair, i) in pend_tp:
                        for p in range(2):
                            pst = psT.tile([128, 128], BF, tag="tp")
                            nc.tensor.transpose(
                                pst,
                                qr_t[:, 2 * p:2 * p + 2, :].rearrange("p a b -> p (a b)"),
                                identb)
                            nc.scalar.copy(
                                out=dst_pair[p][:, i * 128:(i + 1) * 128], in_=pst)
                    pend_tp.clear()

                def qk_path(sl, ns, i, w_sb, wcols, cos_t, sin_t, dst_pair):
                    """Project+norm+rope chunk i of q or k; transpose deferred."""
                    ps = psA.tile([128, 512], F32, tag="proj")
                    for c in range(8):
                        nc.tensor.matmul(ps[:, :wcols],
                                         sl[:, c, ns * 128:(ns + 1) * 128],
                                         w_sb[:, c, :],
                                         start=(c == 0), stop=(c == 7))
                    flush_tp()
                    qpart = ps[:, 0:256]
                    # variance via ACT Square + free-axis accumulate
                    ssum = work.tile([128, 4], F32, tag="ssum")
                    sqj = work.tile([128, 64], F32, tag="sqj")
                    for h in range(4):
                        nc.scalar.activation(
                            out=sqj, in_=qpart[:, h * 64:(h + 1) * 64],
                            func=AF.Square, accum_out=ssum[:, h:h + 1])
                    sdev = work.tile([128, 4], F32, tag="sdev")
                    nc.scalar.activation(out=sdev, in_=ssum, func=AF.Sqrt,
                                         scale=1.0 / 64.0, bias=eps_sb)
                    rstd = work.tile([128, 4], F32, tag="rstd")
                    nc.vector.reciprocal(out=rstd, in_=sdev)
                    # qn = q * rstd (bf16), then rope on DVE in bf16
                    qn = work.tile([128, 4, 64], BF, tag="qn")
                    nc.vector.tensor_tensor(
                        out=qn, in0=qpart.rearrange("p (h d) -> p h d", h=4),
                        in1=bcast_inner(rstd, 64), op=OP.mult)
                    t1 = work.tile([128, 4, 64], BF, tag="t1")
                    nc.vector.tensor_tensor(out=t1, in0=qn, in1=bcast4(cos_t),
                                            op=OP.mult)
                    t2 = work.tile([128, 4, 64], BF, tag="t2")
                    nc.vector.tensor_tensor(out=t2, in0=swap_view(qn),
                                            in1=bcast4(sin_t), op=OP.mult)
                    qr_t = work.tile([128, 4, 64], BF, tag="qr")
                    nc.vector.tensor_tensor(out=qr_t, in0=t1, in1=t2, op=OP.add)
                    pend_tp.append((qr_t, dst_pair, i))
                    return ps

                # K/V path over all 16 chunks
                for qc in range(4):
                    c_sl = slices.tile([128, 8, 512], BF, tag="slice")
                    nc.sync.dma_start(
                        out=c_sl,
                        in_=ctxT.ap()[:, :, qc * 512:(qc + 1) * 512]
                        .rearrange("c p n -> p c n"))
                    for ns in range(4):
                        j = qc * 4 + ns
                        ps = qk_path(c_sl, ns, j, wkv_sb, 512,
                                     ck_sb[:, j, :], sk_sb[:, j, :], pairK)
                        # v evacuation on ACT
                        nc.scalar.copy(
                            out=v_sb[:, j, :, 0:64],
                            in_=ps[:, 256:512].rearrange("p (h d) -> p h d", h=4))

                # Q-side weights / tables now (deferred so K DMAs go first)
                wq_sb = singles.tile([128, 8, 256], BF)
                nc.sync.dma_start(out=wq_sb, in_=wq.ap().rearrange("c p f -> p c f"))
                wg_sb = singles.tile([128, 8, 256], BF)
                nc.sync.dma_start(out=wg_sb, in_=wg.ap().rearrange("c p f -> p c f"))
                cq_sb = singles.tile([128, NCH, D], BF)
                nc.sync.dma_start(out=cq_sb, in_=cosq.ap().rearrange("(i p) d -> p i d", p=128))
                sq_sb = singles.tile([128, NCH, D], BF)
                nc.sync.dma_start(out=sq_sb, in_=ssinq.ap().rearrange("(i p) d -> p i d", p=128))
                wo_sb = singles.tile([128, 2, 1024], BF)
                nc.sync.dma_start(out=wo_sb, in_=wo.ap().rearrange("c p f -> p c f"))

                # Q path + raw gate over all 16 chunks
                for qc in range(4):
                    x_sl = slices.tile([128, 8, 512], BF, tag="slice")
                    nc.sync.dma_start(
                        out=x_sl,
                        in_=xT.ap()[:, :, qc * 512:(qc + 1) * 512]
                        .rearrange("c p n -> p c n"))
                    for ns in range(4):
                        i = qc * 4 + ns
                        qk_path(x_sl, ns, i, wq_sb, 256,
                                cq_sb[:, i, :], sq_sb[:, i, :], pairQ)
                    # gate projection, transposed layout, raw (sigmoid later)
                    for gfc in range(2):
                        psg = psA.tile([128, 512], F32, tag="proj")
                        for c in range(8):
                            nc.tensor.matmul(
                                psg, wg_sb[:, c, gfc * 128:(gfc + 1) * 128],
                                x_sl[:, c, :], start=(c == 0), stop=(c == 7))
                        if gfc == 1:
                            flush_tp()
                        nc.scalar.activation(
                            out=graw[:, gfc, qc * 512:(qc + 1) * 512], in_=psg,
                            func=AF.Copy)
                flush_tp()

            # ================= P2: attention + gating + out-proj =============
            with tc.tile_pool(name="psSC", bufs=2, space="PSUM") as psSC, \
                 tc.tile_pool(name="psAO", bufs=2, space="PSUM") as psAO, \
                 tc.tile_pool(name="psDN", bufs=1, space="PSUM") as psDN:

                oc_jobs = []  # deferred out-proj token-chunks (nk indices)

                def emit_outproj(n1):
                    po = psSC.tile([128, 1024], F32, tag="sc", name="po")
                    for fc in range(2):
                        for oc in range(2):
                            nc.tensor.matmul(
                                po[:, oc * 512:(oc + 1) * 512],
                                A_sb[:, fc, n1 * 128:(n1 + 1) * 128],
                                wo_sb[:, fc, oc * 512:(oc + 1) * 512],
                                start=(fc == 0), stop=(fc == 1))
                    ev = gat_p.tile([128, 1024], BF, tag="ev")
                    nc.vector.tensor_copy(out=ev, in_=po)
                    nc.sync.dma_start(
                        out=part.ap()[n1 * 128:(n1 + 1) * 128, :], in_=ev)

                for qc in range(4):
                    qsl = slice(qc * 512, (qc + 1) * 512)
                    for p in range(2):
                        ao_p = psAO.tile([128, 512], F32, tag="ao")
                        dn_p = psDN.tile([128, 512], F32, tag="dn")
                        pend = []  # (expS tile, ktc) awaiting attn MMs

                        def flush_attn(eS, k, ao_p=ao_p, dn_p=dn_p):
                            # first MM of the k==0 group clears the whole bank;
                            # the second must NOT re-clear (would drop the
                            # first's has_written bits) -> start only on MM1.
                            st = (k == 0)
                            sp = (k == KTC - 1)
                            nc.tensor.matmul(ao_p[0:64, :], v_sb[:, k, 2 * p, :],
                                             eS[:, 0, :], start=st, stop=sp,
                                             tile_position=(0, 0),
                                             skip_group_check=True)
                            nc.tensor.matmul(ao_p[64:128, :], v_sb[:, k, 2 * p + 1, :],
                                             eS[:, 1, :], start=st, stop=sp,
                                             tile_position=(0, 64),
                                             skip_group_check=True)
                            nc.tensor.matmul(dn_p[0:1, :], ones1, eS[:, 0, :],
                                             start=st, stop=sp,
                                             tile_position=(0, 0),
                                             skip_group_check=True)
                            nc.tensor.matmul(dn_p[32:33, :], ones1, eS[:, 1, :],
                                             start=st, stop=sp,
                                             tile_position=(0, 32),
                                             skip_group_check=True)

                        for k in range(KTC):
                            ksl = slice(k * 128, (k + 1) * 128)
                            ps = psSC.tile([128, 1024], F32, tag="sc")
                            nc.tensor.matmul(ps[:, 0:512],
                                             pairK[p][0:64, ksl],
                                             pairQ[p][0:64, qsl],
                                             start=True, stop=True,
                                             tile_position=(0, 0))
                            nc.tensor.matmul(ps[:, 512:1024],
                                             pairK[p][64:128, ksl],
                                             pairQ[p][64:128, qsl],
                                             start=True, stop=True,
                                             tile_position=(64, 0))
                            eS = exps_p.tile([128, 2, 512], BF, tag="expS")
                            nc.scalar.activation(
                                out=eS.rearrange("p a b -> p (a b)"), in_=ps,
                                func=AF.Exp, scale=0.125)
                            pend.append((eS, k))
                            if len(pend) > 2:
                                flush_attn(*pend.pop(0))
                            # out-proj of the previous q block rides the
                            # exp-paced slack so PE never idles long
                            if k in (5, 10) and oc_jobs:
                                emit_outproj(oc_jobs.pop(0))
                        for e in pend:
                            flush_attn(*e)

                        # gating for this pair (sigmoid via exp: same ACT table)
                        gs = gat_p.tile([128, 512], F32, tag="gs")
                        nc.scalar.activation(out=gs, in_=graw[:, p, qsl],
                                             func=AF.Exp, scale=-1.0)
                        dns = gat_p.tile([128, 512], BF, tag="dns")
                        nc.vector.tensor_copy(out=dns[0:1, :], in_=dn_p[0:1, :])
                        nc.vector.tensor_copy(out=dns[32:33, :],
                                              in_=dn_p[32:33, :])
                        rbc = psDN.tile([128, 512], F32, tag="dn", name="rbc")
                        nc.tensor.matmul(rbc[0:64, :], ones2[0:1, :], dns[0:1, :],
                                         start=True, stop=True,
                                         tile_position=(0, 0))
                        nc.tensor.matmul(rbc[64:128, :], ones2[32:33, :],
                                         dns[32:33, :], start=True, stop=True,
                                         tile_position=(32, 64))
                        # w = (1 + e^-g) * dn ; A = ao / w
                        w_t = gat_p.tile([128, 512], F32, tag="w")
                        nc.vector.scalar_tensor_tensor(
                            out=w_t, in0=gs, scalar=1.0, op0=OP.add,
                            in1=rbc, op1=OP.mult)
                        rec = gat_p.tile([128, 512], F32, tag="rec")
                        nc.vector.reciprocal(out=rec, in_=w_t)
                        nc.vector.tensor_tensor(out=A_sb[:, p, qsl], in0=ao_p,
                                                in1=rec, op=OP.mult)

                    oc_jobs.extend(qc * 4 + nk for nk in range(4))

                while oc_jobs:
                    emit_outproj(oc_jobs.pop(0))

    nc.compile()
    return nc


def _prep_core(inputs, b, g, bf16):
    x = np.asarray(inputs["x"][b], dtype=np.float32)
    ctx = np.asarray(inputs["context"][b], dtype=np.float32)
    Wq = np.asarray(inputs["Wq"], dtype=np.float32).reshape(H, 2 * D, C)
    Wkv = np.asarray(inputs["Wkv"], dtype=np.float32).reshape(H, 2 * D, C)
    Wo = np.asarray(inputs["Wo"], dtype=np.float32)
    cos = np.asarray(inputs["cos"][b], dtype=np.float32)
    sin = np.asarray(inputs["sin"][b], dtype=np.float32)
    qw = np.asarray(inputs["q_norm_w"], dtype=np.float32)
    kw = np.asarray(inputs["k_norm_w"], dtype=np.float32)

    hs = slice(HG * g, HG * g + HG)
    qr = Wq[hs, :D, :]                       # [4, D, C]
    qr = qr - qr.mean(axis=1, keepdims=True)
    gr = Wq[hs, D:, :]
    kr = Wkv[hs, :D, :]
    kr = kr - kr.mean(axis=1, keepdims=True)
    vr = Wkv[hs, D:, :]

    sgn = np.where(np.arange(D) < D // 2, -1.0, 1.0).astype(np.float32)
    wswap = lambda w: np.concatenate([w[D // 2:], w[:D // 2]])

    return {
        "xT": np.ascontiguousarray(x.T).reshape(8, 128, N).astype(bf16),
        "ctxT": np.ascontiguousarray(ctx.T).reshape(8, 128, N).astype(bf16),
        "wq": np.ascontiguousarray(qr.reshape(HG * D, C).T).reshape(8, 128, 256).astype(bf16),
        "wg": np.ascontiguousarray(gr.reshape(HG * D, C).T).reshape(8, 128, 256).astype(bf16),
        "wkv": np.ascontiguousarray(
            np.concatenate([kr.reshape(HG * D, C), vr.reshape(HG * D, C)], 0).T
        ).reshape(8, 128, 512).astype(bf16),
        "wo": np.ascontiguousarray(
            Wo[:, 256 * g:256 * (g + 1)].T).reshape(2, 128, C).astype(bf16),
        "cosq": (cos * qw[None, :]).astype(bf16),
        "ssinq": (sin * sgn[None, :] * wswap(qw)[None, :]).astype(bf16),
        "cosk": (cos * kw[None, :]).astype(bf16),
        "ssink": (sin * sgn[None, :] * wswap(kw)[None, :]).astype(bf16),
    }


def kernel(**inputs):
    global _PROG, LAST_EXEC_NS, LAST_PROFILE
    import ml_dtypes
    bf16 = ml_dtypes.bfloat16

    if _PROG is None:
        _PROG = _build_program()
    nc = _PROG

    in_maps = [_prep_core(inputs, core // 4, core % 4, bf16) for core in range(8)]

    trace = bool(os.environ.get("BASS_KERNEL_TRACE"))
    kw = {}
    if trace:
        import types
        from trn_agent_boot.trn_boot import _ntff_profile_via_ctypes
        hook = _ntff_profile_via_ctypes('/opt/axon/libaxon_pjrt.so')
        mod = types.ModuleType('antenv.axon_hooks')
        mod.get_axon_ntff_profile_hook = lambda: hook
        sys.modules['antenv.axon_hooks'] = mod
        from concourse import bass_utils
        bass_utils.upload_artifacts = lambda tmpdir: tmpdir
        kw = dict(trace=True, tmpdir=os.environ.get("BASS_KERNEL_TRACE_DIR"))

    from concourse.bass_utils import run_bass_kernel_spmd
    res = run_bass_kernel_spmd(nc, in_maps, core_ids=list(range(8)), **kw)
    LAST_EXEC_NS = res.exec_time_ns
    LAST_PROFILE = res.profile_json

    bo = np.asarray(inputs["bo"], dtype=np.float32)
    out = np.zeros((B, N, C), dtype=np.float32)
    for core in range(8):
        out[core // 4] += np.asarray(res.results[core]["part"], dtype=np.float32)
    out += bo[None, None, :]
    return out


# revision 9
# speedup vs baseline: 1.2531x; 1.0481x over previous
"""Trainium2 Bass kernel for nn_CrossAttention (B=2, N=2048, C=1024, H=16, D=64).

Sharding: 8 cores = 2 batches x 4 head-groups (4 heads each).
Each core computes its head-group's attention + a partial output projection;
the host sums the 4 partials per batch and adds the bias.

v2 layout (all-bf16 matmul inputs, fp32 PSUM accumulation):
  P1: q/k/v projections (bf16 matmuls), zero-mean folded into host-centered
      weights, variance via ACT Square+accum_out, rstd on DVE, rope on DVE in
      bf16, PE transposes (bf16) into head-paired q^T/k^T tiles with ACT-copy
      evacuation, gate projected in transposed layout (raw, sigmoid deferred).
  P2: per q-block x head-pair: row-tiled paired score matmuls, ACT exp from
      2-bank PSUM (scale=1/8), col-tiled paired attn@v + M=1 ones matmuls for
      softmax denominators, gating via exp-sigmoid (same ACT table as the
      attention exp), reciprocal-denominator broadcast by PE, bf16 output
      projection interleaved into the NEXT q-block's attention to keep PE warm
      (HAM K=8/8), bf16 partial out.
"""

import os
import sys
import numpy as np

for _p in ("/opt/trn_rl_repo", "/opt/pypackages"):
    if _p not in sys.path:
        sys.path.insert(0, _p)

B, N, C = 2, 2048, 1024
H, D = 16, 64
HG = 4            # heads per core
NCH = 16          # token chunks of 128
QB = 4            # q blocks of 512
KTC = 16          # key chunks of 128
EPS = 1e-6

_PROG = None      # cached compiled Bass program
LAST_EXEC_NS = None
LAST_PROFILE = None


def _build_program():
    import concourse.bass as bass
    import concourse.bacc as bacc
    import concourse.tile as tile
    import concourse.mybir as mybir

    F32 = mybir.dt.float32
    BF = mybir.dt.bfloat16
    AF = mybir.ActivationFunctionType
    OP = mybir.AluOpType

    nc = bacc.Bacc("TRN2", target_bir_lowering=False, debug=False, num_devices=8)

    xT = nc.dram_tensor("xT", [8, 128, N], BF, kind="ExternalInput")
    ctxT = nc.dram_tensor("ctxT", [8, 128, N], BF, kind="ExternalInput")
    wq = nc.dram_tensor("wq", [8, 128, 256], BF, kind="ExternalInput")
    wg = nc.dram_tensor("wg", [8, 128, 256], BF, kind="ExternalInput")
    wkv = nc.dram_tensor("wkv", [8, 128, 512], BF, kind="ExternalInput")
    wo = nc.dram_tensor("wo", [2, 128, 1024], BF, kind="ExternalInput")
    cosq = nc.dram_tensor("cosq", [N, D], BF, kind="ExternalInput")
    ssinq = nc.dram_tensor("ssinq", [N, D], BF, kind="ExternalInput")
    cosk = nc.dram_tensor("cosk", [N, D], BF, kind="ExternalInput")
    ssink = nc.dram_tensor("ssink", [N, D], BF, kind="ExternalInput")
    part = nc.dram_tensor("part", [N, C], BF, kind="ExternalOutput")

    def bcast4(ap):
        # [128, 64] -> [128, 4, 64] with step-0 middle dim (read-broadcast)
        return bass.AP(tensor=ap.tensor, offset=ap.offset,
                       ap=[ap.ap[0], [0, 4], ap.ap[1]])

    def bcast_inner(ap, n):
        # [128, 4] -> [128, 4, n] with step-0 inner dim
        return bass.AP(tensor=ap.tensor, offset=ap.offset,
                       ap=[ap.ap[0], ap.ap[1], [0, n]])

    def swap_view(ap):
        # ap: [128, 4, 64] contiguous -> per head read order d+32..d+63, d..d+31
        p, hdim, ddim = ap.ap
        return bass.AP(tensor=ap.tensor, offset=ap.offset + 32 * ddim[0],
                       ap=[p, hdim, [-32 * ddim[0], 2], [ddim[0], 32]])

    with tile.TileContext(nc) as tc:
        import contextlib
        with contextlib.ExitStack() as ctx:
            singles = ctx.enter_context(tc.tile_pool(name="singles", bufs=1))
            slices = ctx.enter_context(tc.tile_pool(name="slices", bufs=2))
            work = ctx.enter_context(tc.tile_pool(name="work", bufs=2))
            persist = ctx.enter_context(tc.tile_pool(name="persist", bufs=1))
            exps_p = ctx.enter_context(tc.tile_pool(name="exps", bufs=6))
            gat_p = ctx.enter_context(tc.tile_pool(name="gat", bufs=2))

            # ---- K-side constants / weights first (needed earliest) ----
            ck_sb = singles.tile([128, NCH, D], BF)
            nc.sync.dma_start(out=ck_sb, in_=cosk.ap().rearrange("(i p) d -> p i d", p=128))
            sk_sb = singles.tile([128, NCH, D], BF)
            nc.sync.dma_start(out=sk_sb, in_=ssink.ap().rearrange("(i p) d -> p i d", p=128))
            wkv_sb = singles.tile([128, 8, 512], BF)
            nc.sync.dma_start(out=wkv_sb, in_=wkv.ap().rearrange("c p f -> p c f"))

            from concourse.masks import make_identity
            identb = singles.tile([128, 128], BF)
            make_identity(nc, identb)
            ones1 = singles.tile([128, 1], BF)
            nc.vector.memset(ones1, 1.0)
            ones2 = singles.tile([128, 64], BF)
            nc.vector.memset(ones2, 1.0)
            eps_sb = singles.tile([128, 1], F32)
            nc.vector.memset(eps_sb, EPS)
            warm = singles.tile([128, 512], BF)
            nc.vector.memset(warm, 0.0)

            # ---- persistent intermediates ----
            pairQ = persist.tile([128, 2, N], BF, tag="pairQ")
            pairK = persist.tile([128, 2, N], BF, tag="pairK")
            v_sb = persist.tile([128, KTC, 4, 64], BF, tag="v_sb")
            sigE = persist.tile([128, 2, N], BF, tag="sigE")   # exp(-gate)
            A_sb = persist.tile([128, 2, N], BF, tag="A_sb")

            # ================= P1: projections / norm / rope / transposes ====
            with tc.tile_pool(name="psA", bufs=3, space="PSUM") as psA, \
                 tc.tile_pool(name="psT", bufs=2, space="PSUM") as psT:

                # PE warmup while input DMAs are in flight: keeps HAM busy so
                # the real matmuls start at K=8/8 (2.4 GHz) instead of 1.2.
                for _ in range(14):
                    wps = psA.tile([128, 512], F32, tag="proj", name="wps")
                    nc.tensor.matmul(wps, warm[:, 0:128], warm,
                                     start=True, stop=True)

                pend_tp = []  # transposes deferred two chunks to keep PE fed

                def flush_tp(keep=0):
                    while len(pend_tp) > keep:
                        qr_t, dst_pair, i = pend_tp.pop(0)
                        pst = psT.tile([128, 256], BF, tag="tp")
                        for p in range(2):
                            nc.tensor.transpose(
                                pst[:, p * 128:(p + 1) * 128],
                                qr_t[:, 2 * p:2 * p + 2, :].rearrange("p a b -> p (a b)"),
                                identb)
                        nc.scalar.copy(
                            out=dst_pair[:, :, i * 128:(i + 1) * 128],
                            in_=pst.rearrange("p (a b) -> p a b", a=2))

                def qk_path(sl, ns, i, w_sb, wcols, cos_t, sin_t, dst_pair):
                    """Project+norm+rope chunk i of q or k; transpose deferred."""
                    ps = psA.tile([128, 512], F32, tag="proj")
                    for c in range(8):
                        nc.tensor.matmul(ps[:, :wcols],
                                         sl[:, c, ns * 128:(ns + 1) * 128],
                                         w_sb[:, c, :],
                                         start=(c == 0), stop=(c == 7))
                    flush_tp(keep=1)
                    qpart = ps[:, 0:256]
                    # variance (zero-mean folded into host-centered weights)
                    sqv = work.tile([128, 256], F32, tag="sq")
                    nc.scalar.activation(out=sqv, in_=qpart, func=AF.Square)
                    ssum = work.tile([128, 4], F32, tag="ssum")
                    nc.vector.tensor_reduce(
                        out=ssum, in_=sqv.rearrange("p (h d) -> p h d", h=4),
                        axis=mybir.AxisListType.X, op=OP.add)
                    sdev = work.tile([128, 4], F32, tag="sdev")
                    nc.scalar.activation(out=sdev, in_=ssum, func=AF.Sqrt,
                                         scale=1.0 / 64.0, bias=eps_sb)
                    rstd = work.tile([128, 4], F32, tag="rstd")
                    nc.vector.reciprocal(out=rstd, in_=sdev)
                    # qn = q * rstd (bf16), then rope on DVE in bf16
                    qn = work.tile([128, 4, 64], BF, tag="qn")
                    nc.vector.tensor_tensor(
                        out=qn, in0=qpart.rearrange("p (h d) -> p h d", h=4),
                        in1=bcast_inner(rstd, 64), op=OP.mult)
                    t1 = work.tile([128, 4, 64], BF, tag="t1")
                    nc.vector.tensor_tensor(out=t1, in0=qn, in1=bcast4(cos_t),
                                            op=OP.mult)
                    t2 = work.tile([128, 4, 64], BF, tag="t2")
                    nc.vector.tensor_tensor(out=t2, in0=swap_view(qn),
                                            in1=bcast4(sin_t), op=OP.mult)
                    qr_t = work.tile([128, 4, 64], BF, tag="qr", bufs=3)
                    nc.vector.tensor_tensor(out=qr_t, in0=t1, in1=t2, op=OP.add)
                    pend_tp.append((qr_t, dst_pair, i))
                    return ps

                # K/V path over all 16 chunks
                for qc in range(4):
                    c_sl = slices.tile([128, 8, 512], BF, tag="slice")
                    nc.sync.dma_start(
                        out=c_sl,
                        in_=ctxT.ap()[:, :, qc * 512:(qc + 1) * 512]
                        .rearrange("c p n -> p c n"))
                    for ns in range(4):
                        j = qc * 4 + ns
                        ps = qk_path(c_sl, ns, j, wkv_sb, 512,
                                     ck_sb[:, j, :], sk_sb[:, j, :], pairK)
                        # v evacuation on ACT
                        nc.scalar.copy(
                            out=v_sb[:, j, :, 0:64],
                            in_=ps[:, 256:512].rearrange("p (h d) -> p h d", h=4))

                # Q-side weights / tables now (deferred so K DMAs go first)
                wq_sb = singles.tile([128, 8, 256], BF)
                nc.sync.dma_start(out=wq_sb, in_=wq.ap().rearrange("c p f -> p c f"))
                wg_sb = singles.tile([128, 8, 256], BF)
                nc.sync.dma_start(out=wg_sb, in_=wg.ap().rearrange("c p f -> p c f"))
                cq_sb = singles.tile([128, NCH, D], BF)
                nc.sync.dma_start(out=cq_sb, in_=cosq.ap().rearrange("(i p) d -> p i d", p=128))
                sq_sb = singles.tile([128, NCH, D], BF)
                nc.sync.dma_start(out=sq_sb, in_=ssinq.ap().rearrange("(i p) d -> p i d", p=128))
                wo_sb = singles.tile([128, 2, 1024], BF)
                nc.sync.dma_start(out=wo_sb, in_=wo.ap().rearrange("c p f -> p c f"))

                # Q path + raw gate over all 16 chunks
                for qc in range(4):
                    x_sl = slices.tile([128, 8, 512], BF, tag="slice")
                    nc.sync.dma_start(
                        out=x_sl,
                        in_=xT.ap()[:, :, qc * 512:(qc + 1) * 512]
                        .rearrange("c p n -> p c n"))
                    for ns in range(4):
                        i = qc * 4 + ns
                        qk_path(x_sl, ns, i, wq_sb, 256,
                                cq_sb[:, i, :], sq_sb[:, i, :], pairQ)
                    # gate projection, transposed layout; evacuate via Exp so
                    # the sigmoid's exp(-g) is precomputed here (P1 ACT slack)
                    for gfc in range(2):
                        psg = psA.tile([128, 512], F32, tag="proj")
                        for c in range(8):
                            nc.tensor.matmul(
                                psg, wg_sb[:, c, gfc * 128:(gfc + 1) * 128],
                                x_sl[:, c, :], start=(c == 0), stop=(c == 7))
                        if gfc == 1:
                            flush_tp(keep=1)
                        nc.scalar.activation(
                            out=sigE[:, gfc, qc * 512:(qc + 1) * 512], in_=psg,
                            func=AF.Exp, scale=-1.0)
                flush_tp()

            # ================= P2: attention + gating + out-proj =============
            with tc.tile_pool(name="psSC", bufs=2, space="PSUM") as psSC, \
                 tc.tile_pool(name="psAO", bufs=2, space="PSUM") as psAO, \
                 tc.tile_pool(name="psDN", bufs=1, space="PSUM") as psDN, \
                 tc.tile_pool(name="psPO", bufs=1, space="PSUM") as psPO:

                oc_jobs = []  # deferred out-proj half-rows (n1, oc)

                def emit_outproj():
                    if not oc_jobs:
                        return
                    n1, oc = oc_jobs.pop(0)
                    po = psPO.tile([128, 512], F32, tag="po")
                    for fc in range(2):
                        nc.tensor.matmul(
                            po,
                            A_sb[:, fc, n1 * 128:(n1 + 1) * 128],
                            wo_sb[:, fc, oc * 512:(oc + 1) * 512],
                            start=(fc == 0), stop=(fc == 1))
                    ev = gat_p.tile([128, 512], BF, tag="ev")
                    nc.vector.tensor_copy(out=ev, in_=po)
                    nc.sync.dma_start(
                        out=part.ap()[n1 * 128:(n1 + 1) * 128,
                                      oc * 512:(oc + 1) * 512], in_=ev)

                for qc in range(4):
                    qsl = slice(qc * 512, (qc + 1) * 512)
                    for p in range(2):
                        ao_p = psAO.tile([128, 512], F32, tag="ao")
                        dn_p = psDN.tile([128, 512], F32, tag="dn")
                        pend = []  # (expS tile, ktc) awaiting attn MMs

                        def flush_attn(eS, k, ao_p=ao_p, dn_p=dn_p):
                            # first MM of the k==0 group clears the whole bank;
                            # the second must NOT re-clear (would drop the
                            # first's has_written bits) -> start only on MM1.
                            st = (k == 0)
                            sp = (k == KTC - 1)
                            nc.tensor.matmul(ao_p[0:64, :], v_sb[:, k, 2 * p, :],
                                             eS[:, 0, :], start=st, stop=sp,
                                             tile_position=(0, 0),
                                             skip_group_check=True)
                            nc.tensor.matmul(ao_p[64:128, :], v_sb[:, k, 2 * p + 1, :],
                                             eS[:, 1, :], start=st, stop=sp,
                                             tile_position=(0, 64),
                                             skip_group_check=True)
                            nc.tensor.matmul(dn_p[0:1, :], ones1, eS[:, 0, :],
                                             start=st, stop=sp,
                                             tile_position=(0, 0),
                                             skip_group_check=True)
                            nc.tensor.matmul(dn_p[32:33, :], ones1, eS[:, 1, :],
                                             start=st, stop=sp,
                                             tile_position=(0, 32),
                                             skip_group_check=True)

                        for k in range(KTC):
                            ksl = slice(k * 128, (k + 1) * 128)
                            ps = psSC.tile([128, 1024], F32, tag="sc")
                            nc.tensor.matmul(ps[:, 0:512],
                                             pairK[0:64, p, ksl],
                                             pairQ[0:64, p, qsl],
                                             start=True, stop=True,
                                             tile_position=(0, 0))
                            nc.tensor.matmul(ps[:, 512:1024],
                                             pairK[64:128, p, ksl],
                                             pairQ[64:128, p, qsl],
                                             start=True, stop=True,
                                             tile_position=(64, 0))
                            eS = exps_p.tile([128, 2, 512], BF, tag="expS")
                            nc.scalar.activation(
                                out=eS.rearrange("p a b -> p (a b)"), in_=ps,
                                func=AF.Exp, scale=0.125)
                            pend.append((eS, k))
                            if len(pend) > 2:
                                flush_attn(*pend.pop(0))
                            # out-proj of the previous q block rides the
                            # exp-paced slack so PE never idles long
                            if k in (2, 5, 8, 11, 14):
                                emit_outproj()
                        for e in pend:
                            flush_attn(*e)

                        # gating: sigmoid(g)/dn with exp(-g) precomputed in P1
                        dns = gat_p.tile([128, 512], BF, tag="dns")
                        nc.vector.tensor_copy(out=dns[0:1, :], in_=dn_p[0:1, :])
                        nc.vector.tensor_copy(out=dns[32:33, :],
                                              in_=dn_p[32:33, :])
                        rbc = psDN.tile([128, 512], F32, tag="dn", name="rbc")
                        nc.tensor.matmul(rbc[0:64, :], ones2[0:1, :], dns[0:1, :],
                                         start=True, stop=True,
                                         tile_position=(0, 0))
                        nc.tensor.matmul(rbc[64:128, :], ones2[32:33, :],
                                         dns[32:33, :], start=True, stop=True,
                                         tile_position=(32, 64))
                        # w = (1 + e^-g) * dn ; A = ao / w
                        w_t = gat_p.tile([128, 512], F32, tag="w")
                        nc.vector.scalar_tensor_tensor(
                            out=w_t, in0=sigE[:, p, qsl], scalar=1.0, op0=OP.add,
                            in1=rbc, op1=OP.mult)
                        rec = gat_p.tile([128, 512], F32, tag="rec")
                        nc.vector.reciprocal_approx_fast(out=rec, in_=w_t)
                        nc.vector.tensor_tensor(out=A_sb[:, p, qsl], in0=ao_p,
                                                in1=rec, op=OP.mult)

                    oc_jobs.extend((qc * 4 + nk, oc)
                                   for nk in range(4) for oc in range(2))

                while oc_jobs:
                    emit_outproj()

    nc.compile()
    return nc


def _prep_core(inputs, b, g, bf16):
    x = np.asarray(inputs["x"][b], dtype=np.float32)
    ctx = np.asarray(inputs["context"][b], dtype=np.float32)
    Wq = np.asarray(inputs["Wq"], dtype=np.float32).reshape(H, 2 * D, C)
    Wkv = np.asarray(inputs["Wkv"], dtype=np.float32).reshape(H, 2 * D, C)
    Wo = np.asarray(inputs["Wo"], dtype=np.float32)
    cos = np.asarray(inputs["cos"][b], dtype=np.float32)
    sin = np.asarray(inputs["sin"][b], dtype=np.float32)
    qw = np.asarray(inputs["q_norm_w"], dtype=np.float32)
    kw = np.asarray(inputs["k_norm_w"], dtype=np.float32)

    hs = slice(HG * g, HG * g + HG)
    qr = Wq[hs, :D, :]                       # [4, D, C]
    qr = qr - qr.mean(axis=1, keepdims=True)
    gr = Wq[hs, D:, :]
    kr = Wkv[hs, :D, :]
    kr = kr - kr.mean(axis=1, keepdims=True)
    vr = Wkv[hs, D:, :]

    sgn = np.where(np.arange(D) < D // 2, -1.0, 1.0).astype(np.float32)
    wswap = lambda w: np.concatenate([w[D // 2:], w[:D // 2]])

    return {
        "xT": np.ascontiguousarray(x.T).reshape(8, 128, N).astype(bf16),
        "ctxT": np.ascontiguousarray(ctx.T).reshape(8, 128, N).astype(bf16),
        "wq": np.ascontiguousarray(qr.reshape(HG * D, C).T).reshape(8, 128, 256).astype(bf16),
        "wg": np.ascontiguousarray(gr.reshape(HG * D, C).T).reshape(8, 128, 256).astype(bf16),
        "wkv": np.ascontiguousarray(
            np.concatenate([kr.reshape(HG * D, C), vr.reshape(HG * D, C)], 0).T
        ).reshape(8, 128, 512).astype(bf16),
        "wo": np.ascontiguousarray(
            Wo[:, 256 * g:256 * (g + 1)].T).reshape(2, 128, C).astype(bf16),
        "cosq": (cos * qw[None, :]).astype(bf16),
        "ssinq": (sin * sgn[None, :] * wswap(qw)[None, :]).astype(bf16),
        "cosk": (cos * kw[None, :]).astype(bf16),
        "ssink": (sin * sgn[None, :] * wswap(kw)[None, :]).astype(bf16),
    }


def kernel(**inputs):
    global _PROG, LAST_EXEC_NS, LAST_PROFILE
    import ml_dtypes
    bf16 = ml_dtypes.bfloat16

    if _PROG is None:
        _PROG = _build_program()
    nc = _PROG

    in_maps = [_prep_core(inputs, core // 4, core % 4, bf16) for core in range(8)]

    trace = bool(os.environ.get("BASS_KERNEL_TRACE"))
    kw = {}
    if trace:
        import types
        from trn_agent_boot.trn_boot import _ntff_profile_via_ctypes
        hook = _ntff_profile_via_ctypes('/opt/axon/libaxon_pjrt.so')
        mod = types.ModuleType('antenv.axon_hooks')
        mod.get_axon_ntff_profile_hook = lambda: hook
        sys.modules['antenv.axon_hooks'] = mod
        from concourse import bass_utils
        bass_utils.upload_artifacts = lambda tmpdir: tmpdir
        kw = dict(trace=True, tmpdir=os.environ.get("BASS_KERNEL_TRACE_DIR"))

    from concourse.bass_utils import run_bass_kernel_spmd
    res = run_bass_kernel_spmd(nc, in_maps, core_ids=list(range(8)), **kw)
    LAST_EXEC_NS = res.exec_time_ns
    LAST_PROFILE = res.profile_json

    bo = np.asarray(inputs["bo"], dtype=np.float32)
    out = np.zeros((B, N, C), dtype=np.float32)
    for core in range(8):
        out[core // 4] += np.asarray(res.results[core]["part"], dtype=np.float32)
    out += bo[None, None, :]
    return out


# revision 12
# speedup vs baseline: 1.5284x; 1.2197x over previous
"""Trainium2 Bass kernel for nn_CrossAttention (B=2, N=2048, C=1024, H=16, D=64).

Sharding: 8 cores = 2 batches x 4 head-groups (4 heads each).
Each core computes its head-group's attention + a partial output projection;
the host sums the 4 partials per batch and adds the bias.

v2 layout (all-bf16 matmul inputs, fp32 PSUM accumulation):
  P1: q/k/v projections (bf16 matmuls), zero-mean folded into host-centered
      weights, variance via ACT Square+accum_out, rstd on DVE, rope on DVE in
      bf16, PE transposes (bf16) into head-paired q^T/k^T tiles with ACT-copy
      evacuation, gate projected in transposed layout (raw, sigmoid deferred).
  P2: per q-block x head-pair: row-tiled paired score matmuls, ACT exp from
      2-bank PSUM (scale=1/8), col-tiled paired attn@v + M=1 ones matmuls for
      softmax denominators, gating via exp-sigmoid (same ACT table as the
      attention exp), reciprocal-denominator broadcast by PE, bf16 output
      projection interleaved into the NEXT q-block's attention to keep PE warm
      (HAM K=8/8), bf16 partial out.
"""

import os
import sys
import numpy as np

for _p in ("/opt/trn_rl_repo", "/opt/pypackages"):
    if _p not in sys.path:
        sys.path.insert(0, _p)

B, N, C = 2, 2048, 1024
H, D = 16, 64
HG = 4            # heads per core
NCH = 16          # token chunks of 128
QB = 4            # q blocks of 512
KTC = 16          # key chunks of 128
EPS = 1e-6

_PROG = None      # cached compiled Bass program
LAST_EXEC_NS = None
LAST_PROFILE = None


def _build_program():
    import concourse.bass as bass
    import concourse.bacc as bacc
    import concourse.tile as tile
    import concourse.mybir as mybir

    F32 = mybir.dt.float32
    BF = mybir.dt.bfloat16
    AF = mybir.ActivationFunctionType
    OP = mybir.AluOpType

    nc = bacc.Bacc("TRN2", target_bir_lowering=False, debug=False, num_devices=8)

    xT = nc.dram_tensor("xT", [8, 128, N], BF, kind="ExternalInput")
    ctxT = nc.dram_tensor("ctxT", [8, 128, N], BF, kind="ExternalInput")
    wq = nc.dram_tensor("wq", [8, 128, 256], BF, kind="ExternalInput")
    wg = nc.dram_tensor("wg", [8, 128, 256], BF, kind="ExternalInput")
    wkv = nc.dram_tensor("wkv", [8, 128, 512], BF, kind="ExternalInput")
    wo = nc.dram_tensor("wo", [2, 128, 1024], BF, kind="ExternalInput")
    cosq = nc.dram_tensor("cosq", [N, D], BF, kind="ExternalInput")
    ssinq = nc.dram_tensor("ssinq", [N, D], BF, kind="ExternalInput")
    cosk = nc.dram_tensor("cosk", [N, D], BF, kind="ExternalInput")
    ssink = nc.dram_tensor("ssink", [N, D], BF, kind="ExternalInput")
    part = nc.dram_tensor("part", [N, C], BF, kind="ExternalOutput")

    def bcast4(ap):
        # [128, 64] -> [128, 4, 64] with step-0 middle dim (read-broadcast)
        return bass.AP(tensor=ap.tensor, offset=ap.offset,
                       ap=[ap.ap[0], [0, 4], ap.ap[1]])

    def bcast_inner(ap, n):
        # [128, 4] -> [128, 4, n] with step-0 inner dim
        return bass.AP(tensor=ap.tensor, offset=ap.offset,
                       ap=[ap.ap[0], ap.ap[1], [0, n]])

    def swap_view(ap):
        # ap: [128, 4, 64] contiguous -> per head read order d+32..d+63, d..d+31
        p, hdim, ddim = ap.ap
        return bass.AP(tensor=ap.tensor, offset=ap.offset + 32 * ddim[0],
                       ap=[p, hdim, [-32 * ddim[0], 2], [ddim[0], 32]])

    with tile.TileContext(nc) as tc:
        import contextlib
        with contextlib.ExitStack() as ctx:
            singles = ctx.enter_context(tc.tile_pool(name="singles", bufs=1))
            slices = ctx.enter_context(tc.tile_pool(name="slices", bufs=2))
            work = ctx.enter_context(tc.tile_pool(name="work", bufs=2))
            persist = ctx.enter_context(tc.tile_pool(name="persist", bufs=1))
            exps_p = ctx.enter_context(tc.tile_pool(name="exps", bufs=6))
            gat_p = ctx.enter_context(tc.tile_pool(name="gat", bufs=2))

            # ---- K-side constants / weights first (needed earliest) ----
            ck_sb = singles.tile([128, NCH, D], BF)
            nc.sync.dma_start(out=ck_sb, in_=cosk.ap().rearrange("(i p) d -> p i d", p=128))
            sk_sb = singles.tile([128, NCH, D], BF)
            nc.sync.dma_start(out=sk_sb, in_=ssink.ap().rearrange("(i p) d -> p i d", p=128))
            wkv_sb = singles.tile([128, 8, 512], BF)
            nc.sync.dma_start(out=wkv_sb, in_=wkv.ap().rearrange("c p f -> p c f"))

            from concourse.masks import make_identity
            identb = singles.tile([128, 128], BF)
            make_identity(nc, identb)
            ones1 = singles.tile([128, 1], BF)
            nc.vector.memset(ones1, 1.0)
            ones2 = singles.tile([128, 64], BF)
            nc.vector.memset(ones2, 1.0)
            eps_sb = singles.tile([128, 1], F32)
            nc.vector.memset(eps_sb, EPS)
            warm = singles.tile([128, 512], BF)
            nc.vector.memset(warm, 0.0)

            # ---- persistent intermediates ----
            pairQ = persist.tile([128, 2, N], BF, tag="pairQ")
            pairK = persist.tile([128, 2, N], BF, tag="pairK")
            v_sb = persist.tile([128, KTC, 4, 64], BF, tag="v_sb")
            graw = persist.tile([128, 2, N], BF, tag="graw")
            sigE = persist.tile([128, 2, N], BF, tag="sigE")   # exp(-gate)
            A_sb = persist.tile([128, 2, N], BF, tag="A_sb")

            # ================= P1: projections / norm / rope / transposes ====
            with tc.tile_pool(name="psA", bufs=3, space="PSUM") as psA, \
                 tc.tile_pool(name="psT", bufs=2, space="PSUM") as psT:

                # PE warmup while input DMAs are in flight: keeps HAM busy so
                # the real matmuls start at K=8/8 (2.4 GHz) instead of 1.2.
                for _ in range(14):
                    wps = psA.tile([128, 512], F32, tag="proj", name="wps")
                    nc.tensor.matmul(wps, warm[:, 0:128], warm,
                                     start=True, stop=True)

                pend_tp = []  # transposes deferred two chunks to keep PE fed

                def flush_tp(keep=0):
                    while len(pend_tp) > keep:
                        qr_t, dst_pair, i = pend_tp.pop(0)
                        pst = psT.tile([128, 256], BF, tag="tp")
                        for p in range(2):
                            nc.tensor.transpose(
                                pst[:, p * 128:(p + 1) * 128],
                                qr_t[:, 2 * p:2 * p + 2, :].rearrange("p a b -> p (a b)"),
                                identb)
                        nc.scalar.copy(
                            out=dst_pair[:, :, i * 128:(i + 1) * 128],
                            in_=pst.rearrange("p (a b) -> p a b", a=2))

                def qk_path(sl, ns, i, w_sb, wcols, cos_t, sin_t, dst_pair):
                    """Project+norm+rope chunk i of q or k; transpose deferred."""
                    ps = psA.tile([128, 512], F32, tag="proj")
                    for c in range(8):
                        nc.tensor.matmul(ps[:, :wcols],
                                         sl[:, c, ns * 128:(ns + 1) * 128],
                                         w_sb[:, c, :],
                                         start=(c == 0), stop=(c == 7))
                    flush_tp(keep=1)
                    qpart = ps[:, 0:256]
                    # variance (zero-mean folded into host-centered weights)
                    sqv = work.tile([128, 256], F32, tag="sq")
                    nc.scalar.activation(out=sqv, in_=qpart, func=AF.Square)
                    ssum = work.tile([128, 4], F32, tag="ssum")
                    nc.vector.tensor_reduce(
                        out=ssum, in_=sqv.rearrange("p (h d) -> p h d", h=4),
                        axis=mybir.AxisListType.X, op=OP.add)
                    sdev = work.tile([128, 4], F32, tag="sdev")
                    nc.scalar.activation(out=sdev, in_=ssum, func=AF.Sqrt,
                                         scale=1.0 / 64.0, bias=eps_sb)
                    rstd = work.tile([128, 4], F32, tag="rstd")
                    nc.vector.reciprocal(out=rstd, in_=sdev)
                    # qn = q * rstd (bf16), then rope on DVE in bf16
                    qn = work.tile([128, 4, 64], BF, tag="qn")
                    nc.vector.tensor_tensor(
                        out=qn, in0=qpart.rearrange("p (h d) -> p h d", h=4),
                        in1=bcast_inner(rstd, 64), op=OP.mult)
                    t1 = work.tile([128, 4, 64], BF, tag="t1")
                    nc.vector.tensor_tensor(out=t1, in0=qn, in1=bcast4(cos_t),
                                            op=OP.mult)
                    t2 = work.tile([128, 4, 64], BF, tag="t2")
                    nc.vector.tensor_tensor(out=t2, in0=swap_view(qn),
                                            in1=bcast4(sin_t), op=OP.mult)
                    qr_t = work.tile([128, 4, 64], BF, tag="qr", bufs=3)
                    nc.vector.tensor_tensor(out=qr_t, in0=t1, in1=t2, op=OP.add)
                    pend_tp.append((qr_t, dst_pair, i))
                    return ps

                # K/V path over all 16 chunks
                for qc in range(4):
                    c_sl = slices.tile([128, 8, 512], BF, tag="slice")
                    nc.sync.dma_start(
                        out=c_sl,
                        in_=ctxT.ap()[:, :, qc * 512:(qc + 1) * 512]
                        .rearrange("c p n -> p c n"))
                    for ns in range(4):
                        j = qc * 4 + ns
                        ps = qk_path(c_sl, ns, j, wkv_sb, 512,
                                     ck_sb[:, j, :], sk_sb[:, j, :], pairK)
                        # v evacuation on ACT
                        nc.scalar.copy(
                            out=v_sb[:, j, :, 0:64],
                            in_=ps[:, 256:512].rearrange("p (h d) -> p h d", h=4))

                # Q-side weights / tables now (deferred so K DMAs go first)
                wq_sb = singles.tile([128, 8, 256], BF)
                nc.sync.dma_start(out=wq_sb, in_=wq.ap().rearrange("c p f -> p c f"))
                wg_sb = singles.tile([128, 8, 256], BF)
                nc.sync.dma_start(out=wg_sb, in_=wg.ap().rearrange("c p f -> p c f"))
                cq_sb = singles.tile([128, NCH, D], BF)
                nc.sync.dma_start(out=cq_sb, in_=cosq.ap().rearrange("(i p) d -> p i d", p=128))
                sq_sb = singles.tile([128, NCH, D], BF)
                nc.sync.dma_start(out=sq_sb, in_=ssinq.ap().rearrange("(i p) d -> p i d", p=128))
                wo_sb = singles.tile([128, 2, 1024], BF)
                nc.sync.dma_start(out=wo_sb, in_=wo.ap().rearrange("c p f -> p c f"))

                # Q path + raw gate over all 16 chunks
                for qc in range(4):
                    x_sl = slices.tile([128, 8, 512], BF, tag="slice")
                    nc.sync.dma_start(
                        out=x_sl,
                        in_=xT.ap()[:, :, qc * 512:(qc + 1) * 512]
                        .rearrange("c p n -> p c n"))
                    for ns in range(4):
                        i = qc * 4 + ns
                        qk_path(x_sl, ns, i, wq_sb, 256,
                                cq_sb[:, i, :], sq_sb[:, i, :], pairQ)
                    # gate projection, transposed layout, raw (Copy keeps the
                    # ACT table set stable; Exp batch happens once at P1 end)
                    for gfc in range(2):
                        psg = psA.tile([128, 512], F32, tag="proj")
                        for c in range(8):
                            nc.tensor.matmul(
                                psg, wg_sb[:, c, gfc * 128:(gfc + 1) * 128],
                                x_sl[:, c, :], start=(c == 0), stop=(c == 7))
                        if gfc == 1:
                            flush_tp(keep=1)
                        nc.scalar.activation(
                            out=graw[:, gfc, qc * 512:(qc + 1) * 512], in_=psg,
                            func=AF.Copy)
                flush_tp()
                # one table switch into the exp set, then it persists into P2
                for gfc in range(2):
                    nc.scalar.activation(out=sigE[:, gfc, :], in_=graw[:, gfc, :],
                                         func=AF.Exp, scale=-1.0)

            # ================= P2: attention + gating + out-proj =============
            with tc.tile_pool(name="psSC", bufs=2, space="PSUM") as psSC, \
                 tc.tile_pool(name="psAO", bufs=2, space="PSUM") as psAO, \
                 tc.tile_pool(name="psDN", bufs=1, space="PSUM") as psDN, \
                 tc.tile_pool(name="psPO", bufs=1, space="PSUM") as psPO:

                oc_jobs = []  # deferred out-proj half-rows (n1, oc)

                def emit_outproj():
                    if not oc_jobs:
                        return
                    n1, oc = oc_jobs.pop(0)
                    po = psPO.tile([128, 512], F32, tag="po")
                    for fc in range(2):
                        nc.tensor.matmul(
                            po,
                            A_sb[:, fc, n1 * 128:(n1 + 1) * 128],
                            wo_sb[:, fc, oc * 512:(oc + 1) * 512],
                            start=(fc == 0), stop=(fc == 1))
                    ev = gat_p.tile([128, 512], BF, tag="ev")
                    nc.vector.tensor_copy(out=ev, in_=po)
                    nc.sync.dma_start(
                        out=part.ap()[n1 * 128:(n1 + 1) * 128,
                                      oc * 512:(oc + 1) * 512], in_=ev)

                for qc in range(4):
                    qsl = slice(qc * 512, (qc + 1) * 512)
                    for p in range(2):
                        ao_p = psAO.tile([128, 512], F32, tag="ao")
                        dn_p = psDN.tile([128, 512], F32, tag="dn")
                        pend = []  # (expS tile, ktc) awaiting attn MMs

                        def flush_attn(eS, k, ao_p=ao_p, dn_p=dn_p):
                            # first MM of the k==0 group clears the whole bank;
                            # the second must NOT re-clear (would drop the
                            # first's has_written bits) -> start only on MM1.
                            st = (k == 0)
                            sp = (k == KTC - 1)
                            nc.tensor.matmul(ao_p[0:64, :], v_sb[:, k, 2 * p, :],
                                             eS[:, 0, :], start=st, stop=sp,
                                             tile_position=(0, 0),
                                             skip_group_check=True)
                            nc.tensor.matmul(ao_p[64:128, :], v_sb[:, k, 2 * p + 1, :],
                                             eS[:, 1, :], start=st, stop=sp,
                                             tile_position=(0, 64),
                                             skip_group_check=True)
                            nc.tensor.matmul(dn_p[0:1, :], ones1, eS[:, 0, :],
                                             start=st, stop=sp,
                                             tile_position=(0, 0),
                                             skip_group_check=True)
                            nc.tensor.matmul(dn_p[32:33, :], ones1, eS[:, 1, :],
                                             start=st, stop=sp,
                                             tile_position=(0, 32),
                                             skip_group_check=True)

                        for k in range(KTC):
                            ksl = slice(k * 128, (k + 1) * 128)
                            ps = psSC.tile([128, 1024], F32, tag="sc")
                            nc.tensor.matmul(ps[:, 0:512],
                                             pairK[0:64, p, ksl],
                                             pairQ[0:64, p, qsl],
                                             start=True, stop=True,
                                             tile_position=(0, 0))
                            nc.tensor.matmul(ps[:, 512:1024],
                                             pairK[64:128, p, ksl],
                                             pairQ[64:128, p, qsl],
                                             start=True, stop=True,
                                             tile_position=(64, 0))
                            eS = exps_p.tile([128, 2, 512], BF, tag="expS")
                            nc.scalar.activation(
                                out=eS.rearrange("p a b -> p (a b)"), in_=ps,
                                func=AF.Exp, scale=0.125)
                            pend.append((eS, k))
                            if len(pend) > 2:
                                flush_attn(*pend.pop(0))
                            # out-proj of the previous q block rides the
                            # exp-paced slack so PE never idles long
                            if k in (2, 5, 8, 11, 14):
                                emit_outproj()
                        for e in pend:
                            flush_attn(*e)

                        # gating: sigmoid(g)/dn with exp(-g) precomputed in P1
                        dns = gat_p.tile([128, 512], BF, tag="dns")
                        nc.vector.tensor_copy(out=dns[0:1, :], in_=dn_p[0:1, :])
                        nc.vector.tensor_copy(out=dns[32:33, :],
                                              in_=dn_p[32:33, :])
                        rbc = psDN.tile([128, 512], F32, tag="dn", name="rbc")
                        nc.tensor.matmul(rbc[0:64, :], ones2[0:1, :], dns[0:1, :],
                                         start=True, stop=True,
                                         tile_position=(0, 0))
                        nc.tensor.matmul(rbc[64:128, :], ones2[32:33, :],
                                         dns[32:33, :], start=True, stop=True,
                                         tile_position=(32, 64))
                        # w = (1 + e^-g) * dn ; A = ao / w
                        w_t = gat_p.tile([128, 512], F32, tag="w")
                        nc.vector.scalar_tensor_tensor(
                            out=w_t, in0=sigE[:, p, qsl], scalar=1.0, op0=OP.add,
                            in1=rbc, op1=OP.mult)
                        rec = gat_p.tile([128, 512], F32, tag="rec")
                        nc.vector.reciprocal_approx_fast(out=rec, in_=w_t)
                        nc.vector.tensor_tensor(out=A_sb[:, p, qsl], in0=ao_p,
                                                in1=rec, op=OP.mult)

                    oc_jobs.extend((qc * 4 + nk, oc)
                                   for nk in range(4) for oc in range(2))

                # tail: the last q block's out-proj can't hide under more
                # attention; batch it through the now-free score banks instead
                seen = set()
                for n1, _ in oc_jobs:
                    if n1 in seen:
                        continue
                    seen.add(n1)
                    po = psSC.tile([128, 1024], F32, tag="sc", name="po")
                    for fc in range(2):
                        for oc in range(2):
                            nc.tensor.matmul(
                                po[:, oc * 512:(oc + 1) * 512],
                                A_sb[:, fc, n1 * 128:(n1 + 1) * 128],
                                wo_sb[:, fc, oc * 512:(oc + 1) * 512],
                                start=(fc == 0), stop=(fc == 1))
                    ev = gat_p.tile([128, 1024], BF, tag="evt")
                    nc.vector.tensor_copy(out=ev, in_=po)
                    nc.sync.dma_start(
                        out=part.ap()[n1 * 128:(n1 + 1) * 128, :], in_=ev)

    nc.compile()
    return nc


def _prep_core(inputs, b, g, bf16):
    x = np.asarray(inputs["x"][b], dtype=np.float32)
    ctx = np.asarray(inputs["context"][b], dtype=np.float32)
    Wq = np.asarray(inputs["Wq"], dtype=np.float32).reshape(H, 2 * D, C)
    Wkv = np.asarray(inputs["Wkv"], dtype=np.float32).reshape(H, 2 * D, C)
    Wo = np.asarray(inputs["Wo"], dtype=np.float32)
    cos = np.asarray(inputs["cos"][b], dtype=np.float32)
    sin = np.asarray(inputs["sin"][b], dtype=np.float32)
    qw = np.asarray(inputs["q_norm_w"], dtype=np.float32)
    kw = np.asarray(inputs["k_norm_w"], dtype=np.float32)

    hs = slice(HG * g, HG * g + HG)
    qr = Wq[hs, :D, :]                       # [4, D, C]
    qr = qr - qr.mean(axis=1, keepdims=True)
    gr = Wq[hs, D:, :]
    kr = Wkv[hs, :D, :]
    kr = kr - kr.mean(axis=1, keepdims=True)
    vr = Wkv[hs, D:, :]

    sgn = np.where(np.arange(D) < D // 2, -1.0, 1.0).astype(np.float32)
    wswap = lambda w: np.concatenate([w[D // 2:], w[:D // 2]])

    return {
        "xT": np.ascontiguousarray(x.T).reshape(8, 128, N).astype(bf16),
        "ctxT": np.ascontiguousarray(ctx.T).reshape(8, 128, N).astype(bf16),
        "wq": np.ascontiguousarray(qr.reshape(HG * D, C).T).reshape(8, 128, 256).astype(bf16),
        "wg": np.ascontiguousarray(gr.reshape(HG * D, C).T).reshape(8, 128, 256).astype(bf16),
        "wkv": np.ascontiguousarray(
            np.concatenate([kr.reshape(HG * D, C), vr.reshape(HG * D, C)], 0).T
        ).reshape(8, 128, 512).astype(bf16),
        "wo": np.ascontiguousarray(
            Wo[:, 256 * g:256 * (g + 1)].T).reshape(2, 128, C).astype(bf16),
        "cosq": (cos * qw[None, :]).astype(bf16),
        "ssinq": (sin * sgn[None, :] * wswap(qw)[None, :]).astype(bf16),
        "cosk": (cos * kw[None, :]).astype(bf16),
        "ssink": (sin * sgn[None, :] * wswap(kw)[None, :]).astype(bf16),
    }


def kernel(**inputs):
    global _PROG, LAST_EXEC_NS, LAST_PROFILE
    import ml_dtypes
    bf16 = ml_dtypes.bfloat16

    if _PROG is None:
        _PROG = _build_program()
    nc = _PROG

    in_maps = [_prep_core(inputs, core // 4, core % 4, bf16) for core in range(8)]

    trace = bool(os.environ.get("BASS_KERNEL_TRACE"))
    kw = {}
    if trace:
        import types
        from trn_agent_boot.trn_boot import _ntff_profile_via_ctypes
        hook = _ntff_profile_via_ctypes('/opt/axon/libaxon_pjrt.so')
        mod = types.ModuleType('antenv.axon_hooks')
        mod.get_axon_ntff_profile_hook = lambda: hook
        sys.modules['antenv.axon_hooks'] = mod
        from concourse import bass_utils
        bass_utils.upload_artifacts = lambda tmpdir: tmpdir
        kw = dict(trace=True, tmpdir=os.environ.get("BASS_KERNEL_TRACE_DIR"))

    from concourse.bass_utils import run_bass_kernel_spmd
    res = run_bass_kernel_spmd(nc, in_maps, core_ids=list(range(8)), **kw)
    LAST_EXEC_NS = res.exec_time_ns
    LAST_PROFILE = res.profile_json

    bo = np.asarray(inputs["bo"], dtype=np.float32)
    out = np.zeros((B, N, C), dtype=np.float32)
    for core in range(8):
        out[core // 4] += np.asarray(res.results[core]["part"], dtype=np.float32)
    out += bo[None, None, :]
    return out
